# revision 7
# baseline (speedup 1.0000x reference)
# Trainium2 Bass kernel for nn_DepParser (BiLSTM dependency parser).
#
# Distribution (8 NeuronCores, SPMD single program, per-core variation only
# through input data):
#   - cores 0-3 run the forward LSTM, cores 4-7 the backward LSTM (backward
#     cores receive the token sequence reversed, so the program is identical).
#   - pairwise AllGather ([[0,4],[1,5],[2,6],[3,7]]) shares the two h-sequences,
#     every core then holds the full [n+1, 2H] LSTM output (transposed layout).
#   - the O(n^2) pair grid is sharded by head-row blocks (one-hot selection
#     matrices as per-core inputs), gold-arc label scores sharded by arc rows.
import numpy as np

N = 256
V = 50000
P = 50
LBL = 40
D = 256
H = 512
NC = 8
NI = N + 1          # 257 rows/cols of M
RB = 33             # i-rows per core (overlapping shards, stride 32)
GB = N // NC        # 32 gold arcs per core

_COMPILED = None
TRACE = False
LAST_RESULTS = None


def _build(nsteps=N):
    import concourse.bass as bass
    import concourse.mybir as mybir
    import concourse.tile as tile
    import concourse.bacc as bacc
    from concourse.masks import make_identity

    F32 = mybir.dt.float32
    U32 = mybir.dt.uint32
    AF = mybir.ActivationFunctionType

    nc = bacc.Bacc("TRN2", target_bir_lowering=False, debug=False, num_devices=NC)

    # ---------------- dram parameters ----------------
    dp = nc.declare_dram_parameter
    words_idx = dp("words_idx", [128, 2], U32, isOutput=False)
    pos_idx = dp("pos_idx", [128, 2], U32, isOutput=False)
    W_emb = dp("W_emb", [V, D], F32, isOutput=False)
    P_emb = dp("P_emb", [P, D], F32, isOutput=False)
    WihT = dp("WihT", [128, 4, 4 * H], F32, isOutput=False)    # [kp, kc, m]
    WhhT = dp("WhhT", [128, 4, 4 * H], F32, isOutput=False)
    b4c = dp("b4c", [128, 16], F32, isOutput=False)
    A1c = dp("A1c", [128, 8, H], F32, isOutput=False)          # fc1 head half, [cp, cc, k]
    A2c = dp("A2c", [128, 8, H], F32, isOutput=False)          # fc1 dep half
    fc1bT = dp("fc1bT", [128, 4], F32, isOutput=False)
    w2T = dp("w2T", [128, 4], F32, isOutput=False)             # fc2 weight chunks
    fc2b_row = dp("fc2b_row", [1, NI], F32, isOutput=False)
    m1aT = dp("m1aT", [128, 8, H], F32, isOutput=False)        # mlp1 head half
    m1bT = dp("m1bT", [128, 8, H], F32, isOutput=False)        # mlp1 dep half
    mlp1bT = dp("mlp1bT", [128, 4], F32, isOutput=False)
    m2T = dp("m2T", [128, 4, LBL], F32, isOutput=False)        # mlp2.T chunks
    mlp2b_row = dp("mlp2b_row", [1, LBL], F32, isOutput=False)
    Esel = dp("Esel", [128, 3, RB], F32, isOutput=False)       # i-shard one-hot
    Egh = dp("Egh", [128, 3, GB], F32, isOutput=False)         # gold head one-hot
    Egd = dp("Egd", [128, 3, GB], F32, isOutput=False)         # gold dep one-hot

    M_out = dp("M_shard", [RB, NI], F32, isOutput=True)
    L_out = dp("L_shard", [GB, LBL], F32, isOutput=True)

    cc_in = nc.dram_tensor("cc_in", [128, 4, NI], F32)
    cc_out = nc.dram_tensor("cc_out", [2, 128, 4, NI], F32)

    IT = [(0, 128), (128, 128), (256, 1)]  # i-tiles over 257

    with tile.TileContext(nc) as tc:
        with (
            tc.tile_pool(name="persist", bufs=1) as pp,
            tc.tile_pool(name="work", bufs=2) as wp,
        ):
            ident = pp.tile([128, 128], F32)
            make_identity(nc, ident[:])
            ones11 = pp.tile([1, 1], F32)
            nc.gpsimd.memset(ones11[:], 1.0)
            ones1g = pp.tile([1, GB], F32)
            nc.gpsimd.memset(ones1g[:], 1.0)

            # ---------------- phase 0: embedding gather + transpose ----------------
            widx = pp.tile([128, 2], U32)
            pidx = pp.tile([128, 2], U32)
            nc.sync.dma_start(widx[:], words_idx[:])
            nc.sync.dma_start(pidx[:], pos_idx[:])

            sbB = tc.tile_pool(name="sbB", bufs=1)
            sbB_pool = sbB.__enter__()
            sbA = tc.tile_pool(name="sbA", bufs=1)
            spA = sbA.__enter__()
            x_nat = spA.tile([128, 2, 2 * D], F32, name="x_nat")   # [tp, tt, feat]
            for tt in range(2):
                nc.gpsimd.indirect_dma_start(
                    out=x_nat[:, tt, 0:D], out_offset=None, in_=W_emb[:],
                    in_offset=bass.IndirectOffsetOnAxis(ap=widx[:, tt : tt + 1], axis=0),
                )
                nc.gpsimd.indirect_dma_start(
                    out=x_nat[:, tt, D : 2 * D], out_offset=None, in_=P_emb[:],
                    in_offset=bass.IndirectOffsetOnAxis(ap=pidx[:, tt : tt + 1], axis=0),
                )

            psA = tc.tile_pool(name="psA", bufs=2, space="PSUM")
            psp = psA.__enter__()
            xT = spA.tile([128, 4, N], F32, name="xT")          # [kp, kc, t]
            for tt in range(2):
                for kc in range(4):
                    trp = psp.tile([128, 128], F32, tag="tr", name="trp")
                    nc.tensor.transpose(
                        trp[:], x_nat[:, tt, 128 * kc : 128 * (kc + 1)], ident[:]
                    )
                    nc.vector.tensor_copy(xT[:, kc, 128 * tt : 128 * (tt + 1)], trp[:])

            # ---------------- phase 1: xg = x @ WihT + bias ----------------
            Wih_sb = spA.tile([128, 4, 4 * H], F32, name="Wih_sb")
            Whh_sb = sbB_pool.tile([128, 4, 4 * H], F32, name="Whh_sb")
            b4_sb = spA.tile([128, 16], F32, name="b4_sb")
            nc.sync.dma_start(Wih_sb[:], WihT[:])
            nc.sync.dma_start(Whh_sb[:], WhhT[:])
            nc.sync.dma_start(b4_sb[:], b4c[:])

            # xgT[p, t, mf] = xg[t, p + 128*mf]  (t-major for per-step [128,16] reads)
            xgT = sbB_pool.tile([128, N, 16], F32, name="xgT")
            for mf in range(16):
                xgp = psp.tile([128, N], F32, tag="xgp", name="xgp")
                for kc in range(4):
                    nc.tensor.matmul(
                        xgp[:],
                        Wih_sb[:, kc, 128 * mf : 128 * (mf + 1)],
                        xT[:, kc, :],
                        start=(kc == 0), stop=(kc == 3),
                    )
                nc.vector.tensor_scalar_add(
                    xgT[:, :, mf], xgp[:], b4_sb[:, mf : mf + 1]
                )

            psA.__exit__(None, None, None)
            sbA.__exit__(None, None, None)
            psB = tc.tile_pool(name="psB", bufs=2, space="PSUM")
            psp = psB.__enter__()
            # ---------------- phase 2: LSTM recurrence ----------------
            hseqT = pp.tile([128, 4, NI], F32)      # [kp, hf, tcol]; col 0 = root zeros
            c_cur = pp.tile([128, 4], F32)
            nc.vector.memset(hseqT[:, :, 0:1], 0.0)
            nc.vector.memset(c_cur[:], 0.0)

            for t in range(nsteps):
                gps = psp.tile([128, 4, 512], F32, tag="gps", name="gps")
                for kc in range(4):
                    for a in range(4):
                        nc.tensor.matmul(
                            gps[32 * a : 32 * a + 1, a, :],
                            hseqT[:, kc, t : t + 1],
                            Whh_sb[:, kc, 512 * a : 512 * (a + 1)],
                            start=(kc == 0), stop=(kc == 3),
                            tile_position=(0, 32 * a),
                        )
                graw = wp.tile([128, 4, 512], F32, tag="graw", name="graw")
                nc.scalar.copy(graw[0:1, 0, :], gps[0:1, 0, :])
                nc.vector.tensor_copy(graw[32:33, 1, :], gps[32:33, 1, :])
                nc.scalar.copy(graw[64:65, 2, :], gps[64:65, 2, :])
                nc.vector.tensor_copy(graw[96:97, 3, :], gps[96:97, 3, :])

                g_sc = wp.tile([128, 16], F32, tag="g_sc", name="g_sc")
                for a in range(4):
                    for v in range(4):
                        nc.sync.dma_start(
                            g_sc[:, 4 * a + v : 4 * a + v + 1],
                            graw[32 * a : 32 * a + 1, a, 128 * v : 128 * (v + 1)],
                        )
                nc.vector.tensor_add(g_sc[:], g_sc[:], xgT[:, t, :])

                sg = wp.tile([128, 16], F32, tag="sg", name="sg")
                tg = wp.tile([128, 4], F32, tag="tg", name="tg")
                nc.scalar.activation(sg[:], g_sc[:], AF.Sigmoid)
                nc.scalar.activation(tg[:], g_sc[:, 8:12], AF.Tanh)

                t1 = wp.tile([128, 4], F32, tag="t1", name="t1")
                nc.vector.tensor_mul(t1[:], sg[:, 0:4], tg[:])
                ct = wp.tile([128, 4], F32, tag="ct", name="ct")
                nc.vector.tensor_mul(ct[:], sg[:, 4:8], c_cur[:])
                nc.vector.tensor_add(c_cur[:], ct[:], t1[:])
                tc2 = wp.tile([128, 4], F32, tag="tc2", name="tc2")
                nc.scalar.activation(tc2[:], c_cur[:], AF.Tanh)
                nc.vector.tensor_mul(hseqT[:, :, t + 1 : t + 2], sg[:, 12:16], tc2[:])

            psB.__exit__(None, None, None)
            sbB.__exit__(None, None, None)
            psC = tc.tile_pool(name="psC", bufs=1, space="PSUM")
            psp = psC.__enter__()
            # ---------------- phase 3: exchange ----------------
            nc.sync.dma_start(cc_in[:], hseqT[:])
            nc.gpsimd.collective_compute(
                "AllGather", mybir.AluOpType.bypass,
                replica_groups=[[0, 4], [1, 5], [2, 6], [3, 7]],
                ins=[cc_in[:]], outs=[cc_out[:]],
            )
            outT = pp.tile([128, 8, NI], F32)       # [cp, cc, i]
            nc.sync.dma_start(outT[:, 0:4, :], cc_out[0][:])
            # backward half arrives time-reversed: col i (>=1) <- src col 257-i
            nc.vector.memset(outT[:, 4:8, 0:1], 0.0)
            for cc in range(4):
                nc.sync.dma_start(
                    outT[:, 4 + cc, 1:NI],
                    cc_out[1][:, cc, 1:NI][:, ::-1],
                )

            # ---------------- phase 4: projections ----------------
            sbC = tc.tile_pool(name="sbC", bufs=1)
            spC = sbC.__enter__()
            A1_sb = spC.tile([128, 8, H], F32, name="A1_sb")
            A2_sb = spC.tile([128, 8, H], F32, name="A2_sb")
            fc1b_sb = pp.tile([128, 4], F32)
            nc.sync.dma_start(A1_sb[:], A1c[:])
            nc.sync.dma_start(A2_sb[:], A2c[:])
            nc.sync.dma_start(fc1b_sb[:], fc1bT[:])

            # s2T[k, j] = sum_c A2[c, k] outT[c, j]  (+ fc1_b)
            s2T = pp.tile([128, 4, NI], F32)
            for kc in range(4):
                s2p = psp.tile([128, NI], F32, tag="s2p", name="s2p")
                for cc in range(8):
                    nc.tensor.matmul(
                        s2p[:],
                        A2_sb[:, cc, 128 * kc : 128 * (kc + 1)],
                        outT[:, cc, :],
                        start=(cc == 0), stop=(cc == 7),
                    )
                nc.vector.tensor_scalar_add(s2T[:, kc, :], s2p[:], fc1b_sb[:, kc : kc + 1])

            # s1nat[i, k] = sum_c outT[c, i] A1[c, k]
            s1nat = spC.tile([128, 3, H], F32, name="s1nat")
            for it, (ib, isz) in enumerate(IT):
                s1p = psp.tile([128, H], F32, tag="s1p", name="s1p")
                for cc in range(8):
                    nc.tensor.matmul(
                        s1p[:isz, :],
                        outT[:, cc, ib : ib + isz],
                        A1_sb[:, cc, :],
                        start=(cc == 0), stop=(cc == 7),
                    )
                nc.vector.tensor_copy(s1nat[:isz, it, :], s1p[:isz, :])

            # s1loc[k, r] = s1 row (i_base + r), via one-hot selection matmul
            Esel_sb = spC.tile([128, 3, RB], F32, name="Esel_sb")
            nc.sync.dma_start(Esel_sb[:], Esel[:])
            s1loc = pp.tile([128, 4, RB], F32)
            for kc in range(4):
                slp = psp.tile([128, RB], F32, tag="slp", name="slp")
                for it, (ib, isz) in enumerate(IT):
                    nc.tensor.matmul(
                        slp[:],
                        s1nat[:isz, it, 128 * kc : 128 * (kc + 1)],
                        Esel_sb[:isz, it, :],
                        start=(it == 0), stop=(it == 2),
                    )
                nc.vector.tensor_copy(s1loc[:, kc, :], slp[:])

            # ---------------- phase 4b: gold-arc label scores ----------------
            m1a_sb = spC.tile([128, 8, H], F32, name="m1a_sb")
            m1b_sb = spC.tile([128, 8, H], F32, name="m1b_sb")
            nc.sync.dma_start(m1a_sb[:], m1aT[:])
            nc.sync.dma_start(m1b_sb[:], m1bT[:])
            U1 = spC.tile([128, 3, H], F32, name="U1")
            U2 = spC.tile([128, 3, H], F32, name="U2")
            for it, (ib, isz) in enumerate(IT):
                u1p = psp.tile([128, H], F32, tag="u1p", name="u1p")
                u2p = psp.tile([128, H], F32, tag="u2p", name="u2p")
                for cc in range(8):
                    nc.tensor.matmul(
                        u1p[:isz, :], outT[:, cc, ib : ib + isz], m1a_sb[:, cc, :],
                        start=(cc == 0), stop=(cc == 7),
                    )
                    nc.tensor.matmul(
                        u2p[:isz, :], outT[:, cc, ib : ib + isz], m1b_sb[:, cc, :],
                        start=(cc == 0), stop=(cc == 7),
                    )
                nc.vector.tensor_copy(U1[:isz, it, :], u1p[:isz, :])
                nc.vector.tensor_copy(U2[:isz, it, :], u2p[:isz, :])

            Egh_sb = spC.tile([128, 3, GB], F32, name="Egh_sb")
            Egd_sb = spC.tile([128, 3, GB], F32, name="Egd_sb")
            m1b_bias = pp.tile([128, 4], F32)
            m2_sb = pp.tile([128, 4, LBL], F32)
            m2b_sb = pp.tile([1, LBL], F32)
            nc.sync.dma_start(Egh_sb[:], Egh[:])
            nc.sync.dma_start(Egd_sb[:], Egd[:])
            nc.sync.dma_start(m1b_bias[:], mlp1bT[:])
            nc.sync.dma_start(m2_sb[:], m2T[:])
            nc.sync.dma_start(m2b_sb[:], mlp2b_row[:])

            # zT[m, g] = sum_i U1[i, m] Egh[i, g] + U2[i, m] Egd[i, g]; tz = tanh(z + b)
            tz = pp.tile([128, 4, GB], F32)
            for mc in range(4):
                zp = psp.tile([128, GB], F32, tag="zp", name="zp")
                for it, (ib, isz) in enumerate(IT):
                    nc.tensor.matmul(
                        zp[:], U1[:isz, it, 128 * mc : 128 * (mc + 1)], Egh_sb[:isz, it, :],
                        start=(it == 0), stop=False,
                    )
                for it, (ib, isz) in enumerate(IT):
                    nc.tensor.matmul(
                        zp[:], U2[:isz, it, 128 * mc : 128 * (mc + 1)], Egd_sb[:isz, it, :],
                        start=False, stop=(it == 2),
                    )
                nc.scalar.activation(
                    tz[:, mc, :], zp[:], AF.Tanh, bias=m1b_bias[:, mc : mc + 1]
                )

            Lp = psp.tile([GB, LBL], F32, tag="Lp", name="Lp")
            for mc in range(4):
                nc.tensor.matmul(
                    Lp[:], tz[:, mc, :], m2_sb[:, mc, :], start=(mc == 0), stop=False,
                )
            nc.tensor.matmul(Lp[:], ones1g[:], m2b_sb[:], start=False, stop=True)
            L_sb = pp.tile([GB, LBL], F32)
            nc.vector.tensor_copy(L_sb[:], Lp[:])
            nc.sync.dma_start(L_out[:], L_sb[:])

            sbC.__exit__(None, None, None)
            psC.__exit__(None, None, None)
            psD = tc.tile_pool(name="psD", bufs=2, space="PSUM")
            psp = psD.__enter__()
            # ---------------- phase 5: pair grid ----------------
            w2_sb = pp.tile([128, 4], F32)
            fc2b_sb = pp.tile([1, NI], F32)
            nc.sync.dma_start(w2_sb[:], w2T[:])
            nc.sync.dma_start(fc2b_sb[:], fc2b_row[:])

            BLK = 8
            r = 0
            while r < RB:
                nb = min(BLK, RB - r)
                xb = wp.tile([128, 4, BLK * NI], F32, tag="xb", name="xb")
                for kb in range(4):
                    for q in range(nb):
                        nc.vector.tensor_scalar_add(
                            xb[:, kb, q * NI : (q + 1) * NI],
                            s2T[:, kb, :],
                            s1loc[:, kb, r + q : r + q + 1],
                        )
                    nc.scalar.activation(
                        xb[:, kb, 0 : nb * NI], xb[:, kb, 0 : nb * NI], AF.Tanh
                    )
                for q in range(nb):
                    mp = psp.tile([1, NI], F32, tag=f"mp{q % 4}", name="mp")
                    for kb in range(4):
                        nc.tensor.matmul(
                            mp[:], w2_sb[:, kb : kb + 1], xb[:, kb, q * NI : (q + 1) * NI],
                            start=(kb == 0), stop=False,
                        )
                    nc.tensor.matmul(mp[:], ones11[:], fc2b_sb[:], start=False, stop=True)
                    mrow = wp.tile([1, NI], F32, tag=f"mrow{q % 4}", name="mrow")
                    if q % 2 == 0:
                        nc.vector.tensor_copy(mrow[:], mp[:])
                    else:
                        nc.scalar.copy(mrow[:], mp[:])
                    nc.sync.dma_start(M_out[r + q : r + q + 1, :], mrow[:])
                r += nb
            psD.__exit__(None, None, None)

    nc.compile()
    return nc


def _prep_inputs(words, pos, gl, W_emb, P_emb, Wih_f, Whh_f, bih_f, bhh_f,
                 Wih_b, Whh_b, bih_b, bhh_b, fc1_w, fc1_b, fc2_w, fc2_b,
                 mlp1_w, mlp1_b, mlp2_w, mlp2_b):
    f32 = np.float32
    words = np.asarray(words).astype(np.uint32)
    pos = np.asarray(pos).astype(np.uint32)
    gl = np.asarray(gl).astype(np.int64)

    def chunkT(w):  # [m, k] weight -> lhsT/rhs chunks [kp, kc, m] of w.T
        k, m = w.shape[1], w.shape[0]
        return np.ascontiguousarray(
            w.T.reshape(k // 128, 128, m).transpose(1, 0, 2)
        ).astype(f32)

    def colsT(v):  # [k] vector -> [128, k//128] column chunks
        return np.ascontiguousarray(v.reshape(-1, 128).T).astype(f32)

    A1 = fc1_w[:, : 2 * H]   # [H, 2H] acts on head
    A2 = fc1_w[:, 2 * H:]
    m1a = mlp1_w[:, : 2 * H]
    m1b = mlp1_w[:, 2 * H:]

    base = {
        "W_emb": np.asarray(W_emb, f32),
        "P_emb": np.asarray(P_emb, f32),
        "A1c": chunkT(A1),
        "A2c": chunkT(A2),
        "fc1bT": colsT(fc1_b),
        "w2T": colsT(fc2_w[0]),
        "fc2b_row": np.full((1, NI), fc2_b[0], f32),
        "m1aT": chunkT(m1a),
        "m1bT": chunkT(m1b),
        "mlp1bT": colsT(mlp1_b),
        "m2T": np.ascontiguousarray(
            mlp2_w.T.reshape(4, 128, LBL).transpose(1, 0, 2)
        ).astype(f32),
        "mlp2b_row": np.asarray(mlp2_b, f32).reshape(1, LBL),
    }

    def onehot(idxs, cols):  # rows 257 -> [128, 3, cols]; pad rows 257..383 zero
        E = np.zeros((384, cols), f32)
        E[np.asarray(idxs, np.int64), np.arange(cols)] = 1.0
        return np.ascontiguousarray(E.reshape(3, 128, cols).transpose(1, 0, 2))

    per_core = []
    for c in range(NC):
        fwd = c < 4
        w = words if fwd else words[::-1]
        p = pos if fwd else pos[::-1]
        Wih, Whh = (Wih_f, Whh_f) if fwd else (Wih_b, Whh_b)
        bias4 = (bih_f + bhh_f) if fwd else (bih_b + bhh_b)
        i_base = min(32 * c, NI - RB)
        g_base = GB * c
        d = dict(base)
        d["words_idx"] = np.ascontiguousarray(w.reshape(2, 128).T.astype(np.uint32))
        d["pos_idx"] = np.ascontiguousarray(p.reshape(2, 128).T.astype(np.uint32))
        d["WihT"] = chunkT(Wih)
        d["WhhT"] = chunkT(Whh)
        d["b4c"] = colsT(bias4)
        d["Esel"] = onehot(np.arange(i_base, i_base + RB), RB)
        d["Egh"] = onehot(gl[g_base : g_base + GB, 0], GB)
        d["Egd"] = onehot(gl[g_base : g_base + GB, 1], GB)
        per_core.append(d)
    return per_core


def kernel(**inputs):
    global _COMPILED
    from concourse.bass_utils import run_bass_kernel_spmd

    if _COMPILED is None:
        _COMPILED = _build()
    nc = _COMPILED

    per_core = _prep_inputs(**inputs)
    res = run_bass_kernel_spmd(nc, per_core, list(range(NC)), trace=TRACE)
    global LAST_RESULTS
    LAST_RESULTS = res

    M = np.zeros((NI, NI), np.float32)
    L = np.zeros((N, LBL), np.float32)
    for c in range(NC):
        i_base = min(32 * c, NI - RB)
        Ms = res.results[c]["M_shard"]
        M[i_base : i_base + RB] = Ms
        L[GB * c : GB * (c + 1)] = res.results[c]["L_shard"]
    return (M, L)


# revision 9
# speedup vs baseline: 2.1657x; 2.1657x over previous
# Trainium2 Bass kernel for nn_DepParser (BiLSTM dependency parser).
#
# Distribution (8 NeuronCores, SPMD single program, per-core variation only
# through input data):
#   - cores 0-3 run the forward LSTM, cores 4-7 the backward LSTM (backward
#     cores receive the token sequence reversed, so the program is identical).
#   - pairwise AllGather ([[0,4],[1,5],[2,6],[3,7]]) shares the two h-sequences,
#     every core then holds the full [n+1, 2H] LSTM output (transposed layout).
#   - the O(n^2) pair grid is sharded by head-row blocks (one-hot selection
#     matrices as per-core inputs), gold-arc label scores sharded by arc rows.
import numpy as np

N = 256
V = 50000
P = 50
LBL = 40
D = 256
H = 512
NC = 8
NI = N + 1          # 257 rows/cols of M
RB = 33             # i-rows per core (overlapping shards, stride 32)
GB = N // NC        # 32 gold arcs per core

_COMPILED = None
TRACE = False
LAST_RESULTS = None


def _build(nsteps=N):
    import concourse.bass as bass
    import concourse.mybir as mybir
    import concourse.tile as tile
    import concourse.bacc as bacc
    from concourse.masks import make_identity

    F32 = mybir.dt.float32
    U32 = mybir.dt.uint32
    AF = mybir.ActivationFunctionType

    nc = bacc.Bacc("TRN2", target_bir_lowering=False, debug=False, num_devices=NC)

    # ---------------- dram parameters ----------------
    dp = nc.declare_dram_parameter
    words_idx = dp("words_idx", [128, 2], U32, isOutput=False)
    pos_idx = dp("pos_idx", [128, 2], U32, isOutput=False)
    W_emb = dp("W_emb", [V, D], F32, isOutput=False)
    P_emb = dp("P_emb", [P, D], F32, isOutput=False)
    WihT = dp("WihT", [128, 4, 4 * H], F32, isOutput=False)    # [kp, kc, m]
    WhhT = dp("WhhT", [128, 4, 4 * H], F32, isOutput=False)
    b4c = dp("b4c", [128, 16], F32, isOutput=False)
    A1c = dp("A1c", [128, 8, H], F32, isOutput=False)          # fc1 head half, [cp, cc, k]
    A2c = dp("A2c", [128, 8, H], F32, isOutput=False)          # fc1 dep half
    fc1bT = dp("fc1bT", [128, 4], F32, isOutput=False)
    w2T = dp("w2T", [128, 4], F32, isOutput=False)             # fc2 weight chunks
    fc2b_row = dp("fc2b_row", [1, NI], F32, isOutput=False)
    m1aT = dp("m1aT", [128, 8, H], F32, isOutput=False)        # mlp1 head half
    m1bT = dp("m1bT", [128, 8, H], F32, isOutput=False)        # mlp1 dep half
    mlp1bT = dp("mlp1bT", [128, 4], F32, isOutput=False)
    m2T = dp("m2T", [128, 4, LBL], F32, isOutput=False)        # mlp2.T chunks
    mlp2b_row = dp("mlp2b_row", [1, LBL], F32, isOutput=False)
    Esel = dp("Esel", [128, 3, RB], F32, isOutput=False)       # i-shard one-hot
    Egh = dp("Egh", [128, 3, GB], F32, isOutput=False)         # gold head one-hot
    Egd = dp("Egd", [128, 3, GB], F32, isOutput=False)         # gold dep one-hot

    M_out = dp("M_shard", [RB, NI], F32, isOutput=True)
    L_out = dp("L_shard", [GB, LBL], F32, isOutput=True)

    cc_in = nc.dram_tensor("cc_in", [128, 4, NI], F32)
    cc_out = nc.dram_tensor("cc_out", [2, 128, 4, NI], F32)

    IT = [(0, 128), (128, 128), (256, 1)]  # i-tiles over 257

    with tile.TileContext(nc) as tc:
        with (
            tc.tile_pool(name="persist", bufs=1) as pp,
            tc.tile_pool(name="work", bufs=2) as wp,
        ):
            ident = pp.tile([128, 128], F32)
            make_identity(nc, ident[:])
            ones11 = pp.tile([1, 1], F32)
            nc.gpsimd.memset(ones11[:], 1.0)
            ones1g = pp.tile([1, GB], F32)
            nc.gpsimd.memset(ones1g[:], 1.0)

            # ---------------- phase 0: embedding gather + transpose ----------------
            widx = pp.tile([128, 2], U32)
            pidx = pp.tile([128, 2], U32)
            nc.sync.dma_start(widx[:], words_idx[:])
            nc.sync.dma_start(pidx[:], pos_idx[:])

            sbB = tc.tile_pool(name="sbB", bufs=1)
            sbB_pool = sbB.__enter__()
            sbA = tc.tile_pool(name="sbA", bufs=1)
            spA = sbA.__enter__()
            x_nat = spA.tile([128, 2, 2 * D], F32, name="x_nat")   # [tp, tt, feat]
            for tt in range(2):
                nc.gpsimd.indirect_dma_start(
                    out=x_nat[:, tt, 0:D], out_offset=None, in_=W_emb[:],
                    in_offset=bass.IndirectOffsetOnAxis(ap=widx[:, tt : tt + 1], axis=0),
                )
                nc.gpsimd.indirect_dma_start(
                    out=x_nat[:, tt, D : 2 * D], out_offset=None, in_=P_emb[:],
                    in_offset=bass.IndirectOffsetOnAxis(ap=pidx[:, tt : tt + 1], axis=0),
                )

            psA = tc.tile_pool(name="psA", bufs=2, space="PSUM")
            psp = psA.__enter__()
            xT = spA.tile([128, 4, N], F32, name="xT")          # [kp, kc, t]
            for tt in range(2):
                for kc in range(4):
                    trp = psp.tile([128, 128], F32, tag="tr", name="trp")
                    nc.tensor.transpose(
                        trp[:], x_nat[:, tt, 128 * kc : 128 * (kc + 1)], ident[:]
                    )
                    nc.vector.tensor_copy(xT[:, kc, 128 * tt : 128 * (tt + 1)], trp[:])

            # ---------------- phase 1: xg = x @ WihT + bias ----------------
            Wih_sb = spA.tile([128, 4, 4 * H], F32, name="Wih_sb")
            Whh_sb = sbB_pool.tile([128, 4, 4 * H], F32, name="Whh_sb")
            b4_sb = spA.tile([128, 16], F32, name="b4_sb")
            nc.sync.dma_start(Wih_sb[:], WihT[:])
            nc.sync.dma_start(Whh_sb[:], WhhT[:])
            nc.sync.dma_start(b4_sb[:], b4c[:])

            # xgT[p, t, mf] = xg[t, p + 128*mf]  (t-major for per-step [128,16] reads)
            xgT = sbB_pool.tile([128, N, 16], F32, name="xgT")
            for mf in range(16):
                xgp = psp.tile([128, N], F32, tag="xgp", name="xgp")
                for kc in range(4):
                    nc.tensor.matmul(
                        xgp[:],
                        Wih_sb[:, kc, 128 * mf : 128 * (mf + 1)],
                        xT[:, kc, :],
                        start=(kc == 0), stop=(kc == 3),
                    )
                nc.vector.tensor_scalar_add(
                    xgT[:, :, mf], xgp[:], b4_sb[:, mf : mf + 1]
                )

            psA.__exit__(None, None, None)
            sbA.__exit__(None, None, None)
            psB = tc.tile_pool(name="psB", bufs=1, space="PSUM")
            psp = psB.__enter__()
            # ---------------- phase 2: LSTM recurrence ----------------
            hseqT = pp.tile([128, 4, NI], F32)      # [kp, hf, tcol]; col 0 = root zeros
            c_cur = pp.tile([128, 4], F32)
            nc.vector.memset(hseqT[:, :, 0:1], 0.0)
            nc.vector.memset(c_cur[:], 0.0)

            for t in range(nsteps):
                gps = psp.tile([128, 4, 512], F32, tag="gps", name="gps")
                for kc in range(4):
                    for a in range(4):
                        nc.tensor.matmul(
                            gps[32 * a : 32 * a + 1, a, :],
                            hseqT[:, kc, t : t + 1],
                            Whh_sb[:, kc, 512 * a : 512 * (a + 1)],
                            start=(kc == 0), stop=(kc == 3),
                            tile_position=(0, 32 * a),
                        )
                # move the 4 strip rows to SBUF (partition-preserving), then
                # PE-transpose the [128,512] row-layout into partition-parallel
                graw = wp.tile([128, 512], F32, tag="graw", name="graw")
                nc.scalar.copy(graw[0:1, :], gps[0:1, 0, :])
                nc.vector.tensor_copy(graw[32:33, :], gps[32:33, 1, :])
                nc.scalar.copy(graw[64:65, :], gps[64:65, 2, :])
                nc.vector.tensor_copy(graw[96:97, :], gps[96:97, 3, :])

                trg = psp.tile([128, 4, 128], F32, tag="trg", name="trg")
                for v in range(4):
                    nc.tensor.transpose(
                        trg[:, v, :], graw[:, 128 * v : 128 * (v + 1)], ident[:]
                    )
                # trg[p, v, 32a] = g[512a + 128v + p]; select cols {0,32,64,96}
                # in (a, v) order -> matches xgT/gate layout col 4a+v
                _t = trg[:, :, :]
                gsel = bass.AP(_t.tensor, _t.offset, [list(_t.ap)[0], [32, 4], [128, 4]])
                gsum = wp.tile([128, 16], F32, tag="gsum", name="gsum")
                nc.vector.tensor_add(gsum[:], gsel, xgT[:, t, :])

                sg = wp.tile([128, 16], F32, tag="sg", name="sg")
                tg = wp.tile([128, 4], F32, tag="tg", name="tg")
                nc.scalar.activation(sg[:], gsum[:], AF.Sigmoid)
                nc.scalar.activation(tg[:], gsum[:, 8:12], AF.Tanh)

                t1 = wp.tile([128, 4], F32, tag="t1", name="t1")
                nc.vector.tensor_mul(t1[:], sg[:, 0:4], tg[:])
                ct = wp.tile([128, 4], F32, tag="ct", name="ct")
                nc.vector.tensor_mul(ct[:], sg[:, 4:8], c_cur[:])
                nc.vector.tensor_add(c_cur[:], ct[:], t1[:])
                tc2 = wp.tile([128, 4], F32, tag="tc2", name="tc2")
                nc.scalar.activation(tc2[:], c_cur[:], AF.Tanh)
                nc.vector.tensor_mul(hseqT[:, :, t + 1 : t + 2], sg[:, 12:16], tc2[:])

            psB.__exit__(None, None, None)
            sbB.__exit__(None, None, None)
            psC = tc.tile_pool(name="psC", bufs=1, space="PSUM")
            psp = psC.__enter__()
            # ---------------- phase 3: exchange ----------------
            nc.sync.dma_start(cc_in[:], hseqT[:])
            nc.gpsimd.collective_compute(
                "AllGather", mybir.AluOpType.bypass,
                replica_groups=[[0, 4], [1, 5], [2, 6], [3, 7]],
                ins=[cc_in[:]], outs=[cc_out[:]],
            )
            outT = pp.tile([128, 8, NI], F32)       # [cp, cc, i]
            nc.sync.dma_start(outT[:, 0:4, :], cc_out[0][:])
            # backward half arrives time-reversed: col i (>=1) <- src col 257-i
            nc.vector.memset(outT[:, 4:8, 0:1], 0.0)
            for cc in range(4):
                nc.sync.dma_start(
                    outT[:, 4 + cc, 1:NI],
                    cc_out[1][:, cc, 1:NI][:, ::-1],
                )

            # ---------------- phase 4: projections ----------------
            sbC = tc.tile_pool(name="sbC", bufs=1)
            spC = sbC.__enter__()
            A1_sb = spC.tile([128, 8, H], F32, name="A1_sb")
            A2_sb = spC.tile([128, 8, H], F32, name="A2_sb")
            fc1b_sb = pp.tile([128, 4], F32)
            nc.sync.dma_start(A1_sb[:], A1c[:])
            nc.sync.dma_start(A2_sb[:], A2c[:])
            nc.sync.dma_start(fc1b_sb[:], fc1bT[:])

            # s2T[k, j] = sum_c A2[c, k] outT[c, j]  (+ fc1_b)
            s2T = pp.tile([128, 4, NI], F32)
            for kc in range(4):
                s2p = psp.tile([128, NI], F32, tag="s2p", name="s2p")
                for cc in range(8):
                    nc.tensor.matmul(
                        s2p[:],
                        A2_sb[:, cc, 128 * kc : 128 * (kc + 1)],
                        outT[:, cc, :],
                        start=(cc == 0), stop=(cc == 7),
                    )
                nc.vector.tensor_scalar_add(s2T[:, kc, :], s2p[:], fc1b_sb[:, kc : kc + 1])

            # s1nat[i, k] = sum_c outT[c, i] A1[c, k]
            s1nat = spC.tile([128, 3, H], F32, name="s1nat")
            for it, (ib, isz) in enumerate(IT):
                s1p = psp.tile([128, H], F32, tag="s1p", name="s1p")
                for cc in range(8):
                    nc.tensor.matmul(
                        s1p[:isz, :],
                        outT[:, cc, ib : ib + isz],
                        A1_sb[:, cc, :],
                        start=(cc == 0), stop=(cc == 7),
                    )
                nc.vector.tensor_copy(s1nat[:isz, it, :], s1p[:isz, :])

            # s1loc[k, r] = s1 row (i_base + r), via one-hot selection matmul
            Esel_sb = spC.tile([128, 3, RB], F32, name="Esel_sb")
            nc.sync.dma_start(Esel_sb[:], Esel[:])
            s1loc = pp.tile([128, 4, RB], F32)
            for kc in range(4):
                slp = psp.tile([128, RB], F32, tag="slp", name="slp")
                for it, (ib, isz) in enumerate(IT):
                    nc.tensor.matmul(
                        slp[:],
                        s1nat[:isz, it, 128 * kc : 128 * (kc + 1)],
                        Esel_sb[:isz, it, :],
                        start=(it == 0), stop=(it == 2),
                    )
                nc.vector.tensor_copy(s1loc[:, kc, :], slp[:])

            # ---------------- phase 4b: gold-arc label scores ----------------
            m1a_sb = spC.tile([128, 8, H], F32, name="m1a_sb")
            m1b_sb = spC.tile([128, 8, H], F32, name="m1b_sb")
            nc.sync.dma_start(m1a_sb[:], m1aT[:])
            nc.sync.dma_start(m1b_sb[:], m1bT[:])
            U1 = spC.tile([128, 3, H], F32, name="U1")
            U2 = spC.tile([128, 3, H], F32, name="U2")
            for it, (ib, isz) in enumerate(IT):
                u1p = psp.tile([128, H], F32, tag="u1p", name="u1p")
                u2p = psp.tile([128, H], F32, tag="u2p", name="u2p")
                for cc in range(8):
                    nc.tensor.matmul(
                        u1p[:isz, :], outT[:, cc, ib : ib + isz], m1a_sb[:, cc, :],
                        start=(cc == 0), stop=(cc == 7),
                    )
                    nc.tensor.matmul(
                        u2p[:isz, :], outT[:, cc, ib : ib + isz], m1b_sb[:, cc, :],
                        start=(cc == 0), stop=(cc == 7),
                    )
                nc.vector.tensor_copy(U1[:isz, it, :], u1p[:isz, :])
                nc.vector.tensor_copy(U2[:isz, it, :], u2p[:isz, :])

            Egh_sb = spC.tile([128, 3, GB], F32, name="Egh_sb")
            Egd_sb = spC.tile([128, 3, GB], F32, name="Egd_sb")
            m1b_bias = pp.tile([128, 4], F32)
            m2_sb = pp.tile([128, 4, LBL], F32)
            m2b_sb = pp.tile([1, LBL], F32)
            nc.sync.dma_start(Egh_sb[:], Egh[:])
            nc.sync.dma_start(Egd_sb[:], Egd[:])
            nc.sync.dma_start(m1b_bias[:], mlp1bT[:])
            nc.sync.dma_start(m2_sb[:], m2T[:])
            nc.sync.dma_start(m2b_sb[:], mlp2b_row[:])

            # zT[m, g] = sum_i U1[i, m] Egh[i, g] + U2[i, m] Egd[i, g]; tz = tanh(z + b)
            tz = pp.tile([128, 4, GB], F32)
            for mc in range(4):
                zp = psp.tile([128, GB], F32, tag="zp", name="zp")
                for it, (ib, isz) in enumerate(IT):
                    nc.tensor.matmul(
                        zp[:], U1[:isz, it, 128 * mc : 128 * (mc + 1)], Egh_sb[:isz, it, :],
                        start=(it == 0), stop=False,
                    )
                for it, (ib, isz) in enumerate(IT):
                    nc.tensor.matmul(
                        zp[:], U2[:isz, it, 128 * mc : 128 * (mc + 1)], Egd_sb[:isz, it, :],
                        start=False, stop=(it == 2),
                    )
                nc.scalar.activation(
                    tz[:, mc, :], zp[:], AF.Tanh, bias=m1b_bias[:, mc : mc + 1]
                )

            Lp = psp.tile([GB, LBL], F32, tag="Lp", name="Lp")
            for mc in range(4):
                nc.tensor.matmul(
                    Lp[:], tz[:, mc, :], m2_sb[:, mc, :], start=(mc == 0), stop=False,
                )
            nc.tensor.matmul(Lp[:], ones1g[:], m2b_sb[:], start=False, stop=True)
            L_sb = pp.tile([GB, LBL], F32)
            nc.vector.tensor_copy(L_sb[:], Lp[:])
            nc.sync.dma_start(L_out[:], L_sb[:])

            sbC.__exit__(None, None, None)
            psC.__exit__(None, None, None)
            psD = tc.tile_pool(name="psD", bufs=2, space="PSUM")
            psp = psD.__enter__()
            # ---------------- phase 5: pair grid ----------------
            w2_sb = pp.tile([128, 4], F32)
            fc2b_sb = pp.tile([1, NI], F32)
            nc.sync.dma_start(w2_sb[:], w2T[:])
            nc.sync.dma_start(fc2b_sb[:], fc2b_row[:])

            BLK = 8
            r = 0
            while r < RB:
                nb = min(BLK, RB - r)
                xb = wp.tile([128, 4, BLK * NI], F32, tag="xb", name="xb")
                for kb in range(4):
                    for q in range(nb):
                        nc.vector.tensor_scalar_add(
                            xb[:, kb, q * NI : (q + 1) * NI],
                            s2T[:, kb, :],
                            s1loc[:, kb, r + q : r + q + 1],
                        )
                    nc.scalar.activation(
                        xb[:, kb, 0 : nb * NI], xb[:, kb, 0 : nb * NI], AF.Tanh
                    )
                for q in range(nb):
                    mp = psp.tile([1, NI], F32, tag=f"mp{q % 4}", name="mp")
                    for kb in range(4):
                        nc.tensor.matmul(
                            mp[:], w2_sb[:, kb : kb + 1], xb[:, kb, q * NI : (q + 1) * NI],
                            start=(kb == 0), stop=False,
                        )
                    nc.tensor.matmul(mp[:], ones11[:], fc2b_sb[:], start=False, stop=True)
                    mrow = wp.tile([1, NI], F32, tag=f"mrow{q % 4}", name="mrow")
                    if q % 2 == 0:
                        nc.vector.tensor_copy(mrow[:], mp[:])
                    else:
                        nc.scalar.copy(mrow[:], mp[:])
                    nc.sync.dma_start(M_out[r + q : r + q + 1, :], mrow[:])
                r += nb
            psD.__exit__(None, None, None)

    nc.compile()
    return nc


def _prep_inputs(words, pos, gl, W_emb, P_emb, Wih_f, Whh_f, bih_f, bhh_f,
                 Wih_b, Whh_b, bih_b, bhh_b, fc1_w, fc1_b, fc2_w, fc2_b,
                 mlp1_w, mlp1_b, mlp2_w, mlp2_b):
    f32 = np.float32
    words = np.asarray(words).astype(np.uint32)
    pos = np.asarray(pos).astype(np.uint32)
    gl = np.asarray(gl).astype(np.int64)

    def chunkT(w):  # [m, k] weight -> lhsT/rhs chunks [kp, kc, m] of w.T
        k, m = w.shape[1], w.shape[0]
        return np.ascontiguousarray(
            w.T.reshape(k // 128, 128, m).transpose(1, 0, 2)
        ).astype(f32)

    def colsT(v):  # [k] vector -> [128, k//128] column chunks
        return np.ascontiguousarray(v.reshape(-1, 128).T).astype(f32)

    A1 = fc1_w[:, : 2 * H]   # [H, 2H] acts on head
    A2 = fc1_w[:, 2 * H:]
    m1a = mlp1_w[:, : 2 * H]
    m1b = mlp1_w[:, 2 * H:]

    base = {
        "W_emb": np.asarray(W_emb, f32),
        "P_emb": np.asarray(P_emb, f32),
        "A1c": chunkT(A1),
        "A2c": chunkT(A2),
        "fc1bT": colsT(fc1_b),
        "w2T": colsT(fc2_w[0]),
        "fc2b_row": np.full((1, NI), fc2_b[0], f32),
        "m1aT": chunkT(m1a),
        "m1bT": chunkT(m1b),
        "mlp1bT": colsT(mlp1_b),
        "m2T": np.ascontiguousarray(
            mlp2_w.T.reshape(4, 128, LBL).transpose(1, 0, 2)
        ).astype(f32),
        "mlp2b_row": np.asarray(mlp2_b, f32).reshape(1, LBL),
    }

    def onehot(idxs, cols):  # rows 257 -> [128, 3, cols]; pad rows 257..383 zero
        E = np.zeros((384, cols), f32)
        E[np.asarray(idxs, np.int64), np.arange(cols)] = 1.0
        return np.ascontiguousarray(E.reshape(3, 128, cols).transpose(1, 0, 2))

    per_core = []
    for c in range(NC):
        fwd = c < 4
        w = words if fwd else words[::-1]
        p = pos if fwd else pos[::-1]
        Wih, Whh = (Wih_f, Whh_f) if fwd else (Wih_b, Whh_b)
        bias4 = (bih_f + bhh_f) if fwd else (bih_b + bhh_b)
        i_base = min(32 * c, NI - RB)
        g_base = GB * c
        d = dict(base)
        d["words_idx"] = np.ascontiguousarray(w.reshape(2, 128).T.astype(np.uint32))
        d["pos_idx"] = np.ascontiguousarray(p.reshape(2, 128).T.astype(np.uint32))
        d["WihT"] = chunkT(Wih)
        d["WhhT"] = chunkT(Whh)
        d["b4c"] = colsT(bias4)
        d["Esel"] = onehot(np.arange(i_base, i_base + RB), RB)
        d["Egh"] = onehot(gl[g_base : g_base + GB, 0], GB)
        d["Egd"] = onehot(gl[g_base : g_base + GB, 1], GB)
        per_core.append(d)
    return per_core


def kernel(**inputs):
    global _COMPILED
    from concourse.bass_utils import run_bass_kernel_spmd

    if _COMPILED is None:
        _COMPILED = _build()
    nc = _COMPILED

    per_core = _prep_inputs(**inputs)
    res = run_bass_kernel_spmd(nc, per_core, list(range(NC)), trace=TRACE)
    global LAST_RESULTS
    LAST_RESULTS = res

    M = np.zeros((NI, NI), np.float32)
    L = np.zeros((N, LBL), np.float32)
    for c in range(NC):
        i_base = min(32 * c, NI - RB)
        Ms = res.results[c]["M_shard"]
        M[i_base : i_base + RB] = Ms
        L[GB * c : GB * (c + 1)] = res.results[c]["L_shard"]
    return (M, L)
